# revision 23
# baseline (speedup 1.0000x reference)
"""Trainium2 Bass kernel for DirectConv2D (3x3 VALID, NCHW/OIHW).

Problem: x [32, 256, 56, 56] int32 (values 0..7 after clip),
         weight [256, 256, 3, 3] fp32 (small non-negative ints 0..6)
         -> out [32, 256, 54, 54] fp32.

Strategy (v2): 1D Winograd F(2,3) along W + direct 3-tap conv along H.
 - Data-parallel across 8 NeuronCores: 4 images per core, weight replicated.
 - Host precomputes the input transform d[pos] (4 planes of ints in [-14,14],
   exact in fp8) and the weight transform w1[kh,pos] = G @ w (halves in
   [-3, 9]; only the value 8.5 rounds in e4m3 -> max rel err ~5e-3 measured
   against the reference on the real data, well under the 2e-2 gate).
 - M[pos] = sum_kh W1[kh,pos]^T @ d[pos](rows r+kh): fp8 DoubleRow matmuls
   (256-channel contraction per pass), 3-tap PSUM accumulation.
   Per core: 4 img x 2 oc x 3 row-chunks x 4 pos x 3 kh = 288 matmuls of
   486 moving cols (vs 432 for direct conv: 1.5x fewer PE cycles).
 - Output transform on-chip: out_even = M0+M1+M2, out_odd = M1-M2-M3,
   computed with ACT copies (PSUM->SBUF) + DVE tensor_tensor ops (each DVE
   op reads at most ONE PSUM operand - PSUM has a single DVE read port),
   writing bf16 interleaved columns directly (abs err <= ~137 vs outputs
   >= 21k). Host upcasts the bf16 result to fp32.
"""

import sys

sys.path.insert(0, "/opt/trn_rl_repo")

import ml_dtypes
import numpy as np

N_CORES = 8
IMGS = 4  # images per core
H = W = 56
OH = OW = 54
TC = 27  # col tiles (2 output cols each)
ROWS_PER_CHUNK = 18
N_CHUNKS = OH // ROWS_PER_CHUNK  # 3
NT = ROWS_PER_CHUNK * TC  # 486 (<= 512 fp32 PSUM bank)
DPIX = H * TC  # 1512 per (c, pos, img) plane

_PROGRAM_CACHE = {}


def _build_program(mode="fp8dr"):
    import concourse.bacc as bacc
    import concourse.mybir as mybir
    import concourse.tile as tile

    nc = bacc.Bacc(
        "TRN2",
        target_bir_lowering=False,
        debug=False,
        enable_asserts=False,
        num_devices=N_CORES,
    )
    dt8 = mybir.dt.float8e4
    dtb = mybir.dt.bfloat16
    dt_in = dt8 if mode == "fp8dr" else dtb
    fp32 = mybir.dt.float32

    # d planes: [ki, c, pos, img, h*tc]
    d_d = nc.dram_tensor("x_sb", [128, 2, 4, IMGS, DPIX], dt_in, kind="ExternalInput").ap()
    # transformed weights: [ki, kh, pos, oc, c, m]
    w_d = nc.dram_tensor("w_sb", [128, 3, 4, 2, 2, 128], dt_in, kind="ExternalInput").ap()
    out_d = nc.dram_tensor(
        "out", [IMGS, 256, OH, OW], dtb, kind="ExternalOutput"
    ).ap()

    with tile.TileContext(nc) as tc:
        with (
            tc.tile_pool(name="const", bufs=1) as const_pool,
            tc.tile_pool(name="psum", bufs=2, space="PSUM") as psum_pool,
            tc.tile_pool(name="tmp", bufs=8) as tmp_pool,
            tc.tile_pool(name="outs", bufs=4) as out_pool,
        ):
            # PE warm-up on scratch during the input-load window (HAM
            # un-throttle). fp8 build: minimal memset so the warm-up has no
            # real dependencies; bf16 build zeroes for CoreSim.
            w_warm = const_pool.tile([128, 2, 128], dt_in)
            x_warm = const_pool.tile([128, 2, 544], dt_in)
            if mode != "fp8dr":
                nc.gpsimd.memset(w_warm, 0.0)
                nc.gpsimd.memset(x_warm, 0.0)
            else:
                nc.gpsimd.memset(w_warm[:, 0, 0:2], 0.0)
                nc.gpsimd.memset(x_warm[:, 0, 0:2], 0.0)
            pt_warm_4 = psum_pool.tile([128, 4, 512], fp32, tag="pt")
            pt_warm = pt_warm_4[:, 0, 0:NT]
            # warm-up sized to bridge PE from ~7.4us (first issue) to ~13.5us
            # (first input DMA semaphore): ~8 cold MMs at 405ns until HAM
            # fires (~10.9us), then ~246ns each, keeping HAM activity
            # continuous into the real stream
            N_WARM = 26
            for i in range(N_WARM):
                rhs_w = x_warm[:, :, 0:NT]
                if mode == "fp8dr":
                    nc.tensor.matmul(
                        pt_warm, w_warm, rhs_w,
                        start=(i == 0), stop=(i == N_WARM - 1),
                        perf_mode=mybir.MatmulPerfMode.DoubleRow,
                    )
                else:
                    nc.tensor.matmul(
                        pt_warm, w_warm[:, 0], rhs_w[:, 0],
                        start=(i == 0), stop=(i == N_WARM - 1),
                    )

            wt = const_pool.tile([128, 3, 4, 2, 2, 128], dt_in)
            # per-image d tiles: [ki, c, pos, h*tc]
            dts = [
                const_pool.tile([128, 2, 4, DPIX], dt_in, name=f"dt{n}", tag=f"dt{n}")
                for n in range(IMGS)
            ]
            # First MM group needs: wt oc0 (all kh/pos) + dt0 rows 0..19.
            # Split the critical bytes across both rings, then order the rest
            # by first use.
            # Lead covers chunks 0-1 (rows 0..37) so the first two MM groups
            # (~6us of PE work) run while the rests stream in. The two rests
            # go on DIFFERENT rings (serial rests starved the PE by ~7us).
            # wt-oc1 is first needed at ~24us, after d0-c1's rest.
            LEAD = 38 * TC
            nc.sync.dma_start(out=wt[:, :, :, 0], in_=w_d[:, :, :, 0])
            nc.sync.dma_start(out=dts[0][:, 0, :, 0:LEAD], in_=d_d[:, 0, :, 0, 0:LEAD])
            nc.sync.dma_start(out=dts[0][:, 0, :, LEAD:], in_=d_d[:, 0, :, 0, LEAD:])
            nc.scalar.dma_start(out=dts[0][:, 1, :, 0:LEAD], in_=d_d[:, 1, :, 0, 0:LEAD])
            nc.scalar.dma_start(out=dts[0][:, 1, :, LEAD:], in_=d_d[:, 1, :, 0, LEAD:])
            nc.scalar.dma_start(out=wt[:, :, :, 1], in_=w_d[:, :, :, 1])
            # remaining images: interleave rings, ordered by first use
            for n in range(1, IMGS):
                nc.sync.dma_start(out=dts[n][:, 0], in_=d_d[:, 0, :, n])
                nc.scalar.dma_start(out=dts[n][:, 1], in_=d_d[:, 1, :, n])

            n_group = 0
            N_GROUPS = IMGS * 2 * N_CHUNKS
            for n in range(IMGS):
                for oc in range(2):
                    for ch in range(N_CHUNKS):
                        h0 = ch * ROWS_PER_CHUNK
                        # One 4-bank PSUM tile per group: bank pos holds
                        # M[pos]. pos order (1,2,0,3): M1/M2 finish first so
                        # the ACT copies + GPSIMD subtract run under the
                        # remaining MMs; after the last MM only one DVE op
                        # remains.
                        pt4 = psum_pool.tile([128, 4, 512], fp32, tag="pt")
                        pts = [pt4[:, pos, 0:NT] for pos in range(4)]
                        for pos in (1, 2, 0, 3):
                            pt = pts[pos]
                            for kh in range(3):
                                off = (h0 + kh) * TC
                                # winograd rhs windows are fully contiguous:
                                # flat 486-elem inner dim streams best
                                if mode == "fp8dr":
                                    rhs = dts[n][:, :, pos, off : off + NT]
                                    nc.tensor.matmul(
                                        pt,
                                        wt[:, kh, pos, oc],
                                        rhs,
                                        start=(kh == 0),
                                        stop=(kh == 2),
                                        perf_mode=mybir.MatmulPerfMode.DoubleRow,
                                    )
                                else:
                                    for c in range(2):
                                        rhs = dts[n][:, c, pos, off : off + NT]
                                        nc.tensor.matmul(
                                            pt,
                                            wt[:, kh, pos, oc, c],
                                            rhs,
                                            start=(kh == 0 and c == 0),
                                            stop=(kh == 2 and c == 1),
                                        )
                        # output transform: even cols = M0+M1+M2 as a single
                        # DVE tensor_reduce over banks 0..2 (innermost axis
                        # hops banks, stride 512); odd cols = M1-M2-M3 via
                        # ACT copies (PSUM->SBUF), GPSIMD SBUF subtract, and
                        # one DVE op reading one PSUM bank.
                        ot = out_pool.tile([128, ROWS_PER_CHUNK, OW], dtb)
                        t1 = tmp_pool.tile([128, NT], fp32, bufs=12)
                        t2 = tmp_pool.tile([128, NT], fp32, bufs=12)
                        u = tmp_pool.tile([128, NT], fp32)
                        nc.scalar.copy(t1, pts[1])
                        nc.scalar.copy(t2, pts[2])
                        nc.gpsimd.tensor_tensor(u, t1, t2, mybir.AluOpType.subtract)
                        with nc.allow_low_precision(
                            reason="3-term sum of exact ints; bf16 out is "
                            "within the validated 5e-3 error budget"
                        ):
                            nc.vector.tensor_reduce(
                                ot[:, :, 0::2].rearrange("p h w -> p (h w)"),
                                pt4[:, 0:3, 0:NT].rearrange("p c x -> p x c"),
                                mybir.AxisListType.X,
                                mybir.AluOpType.add,
                            )
                        nc.vector.tensor_tensor(
                            ot[:, :, 1::2].rearrange("p h w -> p (h w)"),
                            u, pts[3], mybir.AluOpType.subtract,
                        )
                        n_group += 1
                        last = n_group == N_GROUPS
                        if last:
                            # split the final store across both rings so the
                            # completion tail is short
                            s = ROWS_PER_CHUNK // 2
                            nc.sync.dma_start(
                                out=out_d[n, oc * 128 : (oc + 1) * 128, h0 : h0 + s, :],
                                in_=ot[:, 0:s, :],
                            )
                            nc.scalar.dma_start(
                                out=out_d[n, oc * 128 : (oc + 1) * 128,
                                          h0 + s : h0 + ROWS_PER_CHUNK, :],
                                in_=ot[:, s:, :],
                            )
                        else:
                            # sync ring is idle after the early loads; keep
                            # stores off scalar so the ACT sequencer only
                            # runs the transform copies
                            nc.sync.dma_start(
                                out=out_d[n, oc * 128 : (oc + 1) * 128,
                                          h0 : h0 + ROWS_PER_CHUNK, :],
                                in_=ot,
                            )
    nc.compile()
    return nc


def get_program(mode="fp8dr"):
    if mode not in _PROGRAM_CACHE:
        _PROGRAM_CACHE[mode] = _build_program(mode)
    return _PROGRAM_CACHE[mode]


def _np_dtype(mode):
    return ml_dtypes.float8_e4m3 if mode == "fp8dr" else ml_dtypes.bfloat16


def prep_weight(weight, mode="fp8dr"):
    """weight [256,256,3,3] OIHW fp32 -> w_sb [128 ki, 3 kh, 4 pos, 2 oc, 2 c, 128 m].

    w1[o,i,kh,pos] = sum_kw G[pos,kw] w[o,i,kh,kw], G = F(2,3) weight transform.
    """
    G = np.array([[1, 0, 0], [0.5, 0.5, 0.5], [0.5, -0.5, 0.5], [0, 0, 1]], np.float32)
    wq = weight.astype(np.int32).astype(np.float32)
    w1 = np.einsum("pk,oihk->oihp", G, wq)  # [o, i, kh, pos]
    w1 = w1.reshape(2, 128, 2, 128, 3, 4)  # [oc, m, c, ki, kh, pos]
    w_sb = np.ascontiguousarray(w1.transpose(3, 4, 5, 0, 2, 1))  # [ki, kh, pos, oc, c, m]
    return w_sb.astype(_np_dtype(mode))


def prep_x_core(x_core, mode="fp8dr"):
    """x_core [IMGS, 256, 56, 56] int32 -> d_sb [128 ki, 2 c, 4 pos, IMGS, 56*27]."""
    xq = np.clip(x_core.astype(np.int32), 0, 7).astype(np.float32)
    xq = xq.reshape(IMGS, 2, 128, H, W)  # [n, c, ki, h, w]
    d0 = xq[..., 0:54:2] - xq[..., 2:56:2]
    d1 = xq[..., 1:55:2] + xq[..., 2:56:2]
    d2 = xq[..., 2:56:2] - xq[..., 1:55:2]
    d3 = xq[..., 1:55:2] - xq[..., 3:56:2]
    d = np.stack([d0, d1, d2, d3], axis=0)  # [pos, n, c, ki, h, tc]
    d_sb = np.ascontiguousarray(d.transpose(3, 2, 0, 1, 4, 5))  # [ki, c, pos, n, h, tc]
    return d_sb.reshape(128, 2, 4, IMGS, DPIX).astype(_np_dtype(mode))


def make_in_maps(x, weight, mode="fp8dr"):
    w_sb = prep_weight(weight, mode)
    return [
        {"x_sb": prep_x_core(x[c * IMGS : (c + 1) * IMGS], mode), "w_sb": w_sb}
        for c in range(N_CORES)
    ]


def kernel(x, weight):
    import time

    from concourse.bass_utils import run_bass_kernel_spmd

    mode = "fp8dr"
    nc = get_program(mode)
    in_maps = make_in_maps(np.asarray(x), np.asarray(weight), mode)
    last_err = None
    for attempt in range(3):
        try:
            res = run_bass_kernel_spmd(nc, in_maps, list(range(N_CORES)))
            break
        except Exception as e:  # transient NRT_EXEC_UNIT_UNRECOVERABLE flakes
            last_err = e
            time.sleep(2.0)
    else:
        raise last_err
    return np.concatenate(
        [res.results[c]["out"] for c in range(N_CORES)], axis=0
    ).astype(np.float32)


# revision 26
# speedup vs baseline: 1.7168x; 1.7168x over previous
"""Trainium2 Bass kernel for DirectConv2D (3x3 VALID, NCHW/OIHW).

Problem: x [32, 256, 56, 56] int32 (values 0..7 after clip),
         weight [256, 256, 3, 3] fp32 (small non-negative ints 0..6)
         -> out [32, 256, 54, 54] fp32.

Strategy (v2): 1D Winograd F(2,3) along W + direct 3-tap conv along H.
 - Data-parallel across 8 NeuronCores: 4 images per core, weight replicated.
 - Host precomputes the input transform d[pos] (4 planes of ints in [-14,14],
   exact in fp8) and the weight transform w1[kh,pos] = G @ w (halves in
   [-3, 9]; only the value 8.5 rounds in e4m3 -> max rel err ~5e-3 measured
   against the reference on the real data, well under the 2e-2 gate).
 - M[pos] = sum_kh W1[kh,pos]^T @ d[pos](rows r+kh): fp8 DoubleRow matmuls
   (256-channel contraction per pass), 3-tap PSUM accumulation.
   Per core: 4 img x 2 oc x 3 row-chunks x 4 pos x 3 kh = 288 matmuls of
   486 moving cols (vs 432 for direct conv: 1.5x fewer PE cycles).
 - Output transform on-chip: out_even = M0+M1+M2, out_odd = M1-M2-M3,
   computed with ACT copies (PSUM->SBUF) + DVE tensor_tensor ops (each DVE
   op reads at most ONE PSUM operand - PSUM has a single DVE read port),
   writing bf16 interleaved columns directly (abs err <= ~137 vs outputs
   >= 21k). Host upcasts the bf16 result to fp32.
"""

import sys

sys.path.insert(0, "/opt/trn_rl_repo")

import ml_dtypes
import numpy as np

N_CORES = 8
IMGS = 4  # images per core
H = W = 56
OH = OW = 54
TC = 27  # col tiles (2 output cols each)
ROWS_PER_CHUNK = 18
N_CHUNKS = OH // ROWS_PER_CHUNK  # 3
NT = ROWS_PER_CHUNK * TC  # 486 (<= 512 fp32 PSUM bank)
DPIX = H * TC  # 1512 per (c, pos, img) plane

_PROGRAM_CACHE = {}


def _build_program(mode="fp8dr"):
    import concourse.bacc as bacc
    import concourse.mybir as mybir
    import concourse.tile as tile

    nc = bacc.Bacc(
        "TRN2",
        target_bir_lowering=False,
        debug=False,
        enable_asserts=False,
        num_devices=N_CORES,
    )
    dt8 = mybir.dt.float8e4
    dtb = mybir.dt.bfloat16
    dt_in = dt8 if mode == "fp8dr" else dtb
    fp32 = mybir.dt.float32

    # d planes: [ki, c, pos, img, h*tc]
    d_d = nc.dram_tensor("x_sb", [128, 2, 4, IMGS, DPIX], dt_in, kind="ExternalInput").ap()
    # transformed weights: [ki, kh, pos, oc, c, m]
    w_d = nc.dram_tensor("w_sb", [128, 3, 4, 2, 2, 128], dt_in, kind="ExternalInput").ap()
    out_d = nc.dram_tensor(
        "out", [IMGS, 256, OH, OW], dtb, kind="ExternalOutput"
    ).ap()

    with tile.TileContext(nc) as tc:
        with (
            tc.tile_pool(name="const", bufs=1) as const_pool,
            tc.tile_pool(name="psum", bufs=8, space="PSUM") as psum_pool,
            tc.tile_pool(name="tmp", bufs=8) as tmp_pool,
            tc.tile_pool(name="outs", bufs=4) as out_pool,
        ):
            # PE warm-up on scratch during the input-load window (HAM
            # un-throttle). fp8 build: minimal memset so the warm-up has no
            # real dependencies; bf16 build zeroes for CoreSim.
            w_warm = const_pool.tile([128, 2, 128], dt_in)
            x_warm = const_pool.tile([128, 2, 544], dt_in)
            if mode != "fp8dr":
                nc.gpsimd.memset(w_warm, 0.0)
                nc.gpsimd.memset(x_warm, 0.0)
            else:
                nc.gpsimd.memset(w_warm[:, 0, 0:2], 0.0)
                nc.gpsimd.memset(x_warm[:, 0, 0:2], 0.0)
            pt_warm = psum_pool.tile([128, NT], fp32, tag="pt")
            # warm-up sized to bridge PE from ~7.4us (first issue) to ~13.5us
            # (first input DMA semaphore): ~8 cold MMs at 405ns until HAM
            # fires (~10.9us), then ~246ns each, keeping HAM activity
            # continuous into the real stream
            N_WARM = 26
            for i in range(N_WARM):
                rhs_w = x_warm[:, :, 0:NT]
                if mode == "fp8dr":
                    nc.tensor.matmul(
                        pt_warm, w_warm, rhs_w,
                        start=(i == 0), stop=(i == N_WARM - 1),
                        perf_mode=mybir.MatmulPerfMode.DoubleRow,
                    )
                else:
                    nc.tensor.matmul(
                        pt_warm, w_warm[:, 0], rhs_w[:, 0],
                        start=(i == 0), stop=(i == N_WARM - 1),
                    )

            wt = const_pool.tile([128, 3, 4, 2, 2, 128], dt_in)
            # per-image d tiles: [ki, c, pos, h*tc]
            dts = [
                const_pool.tile([128, 2, 4, DPIX], dt_in, name=f"dt{n}", tag=f"dt{n}")
                for n in range(IMGS)
            ]
            # First MM group needs: wt oc0 (all kh/pos) + dt0 rows 0..19.
            # Split the critical bytes across both rings, then order the rest
            # by first use.
            # Lead covers chunks 0-1 (rows 0..37) so the first two MM groups
            # (~6us of PE work) run while the rests stream in. The two rests
            # go on DIFFERENT rings (serial rests starved the PE by ~7us).
            # wt-oc1 is first needed at ~24us, after d0-c1's rest.
            LEAD = 38 * TC
            nc.sync.dma_start(out=wt[:, :, :, 0], in_=w_d[:, :, :, 0])
            nc.sync.dma_start(out=dts[0][:, 0, :, 0:LEAD], in_=d_d[:, 0, :, 0, 0:LEAD])
            nc.sync.dma_start(out=dts[0][:, 0, :, LEAD:], in_=d_d[:, 0, :, 0, LEAD:])
            nc.scalar.dma_start(out=dts[0][:, 1, :, 0:LEAD], in_=d_d[:, 1, :, 0, 0:LEAD])
            nc.scalar.dma_start(out=dts[0][:, 1, :, LEAD:], in_=d_d[:, 1, :, 0, LEAD:])
            nc.scalar.dma_start(out=wt[:, :, :, 1], in_=w_d[:, :, :, 1])
            # remaining images: interleave rings, ordered by first use
            for n in range(1, IMGS):
                nc.sync.dma_start(out=dts[n][:, 0], in_=d_d[:, 0, :, n])
                nc.scalar.dma_start(out=dts[n][:, 1], in_=d_d[:, 1, :, n])

            n_group = 0
            N_GROUPS = IMGS * 2 * N_CHUNKS
            for n in range(IMGS):
                for oc in range(2):
                    for ch in range(N_CHUNKS):
                        h0 = ch * ROWS_PER_CHUNK
                        # pos order (1,2,0,3) with the output transform ops
                        # issued IN PROGRAM ORDER right after the MMs they
                        # read: Tile emits semaphore watermarks that enforce
                        # its static schedule, so ops placed after all 12 MMs
                        # get pinned a whole group late (measured 2.3us PE
                        # stall per group). even = M0+M1+M2, odd = M1-M2-M3;
                        # each DVE op reads exactly one PSUM tile (single DVE
                        # PSUM port), ACT stages M1/M2, GPSIMD does the
                        # SBUF-only subtract.
                        ot = out_pool.tile([128, ROWS_PER_CHUNK, OW], dtb)
                        t1 = tmp_pool.tile([128, NT], fp32, bufs=12)
                        t2 = tmp_pool.tile([128, NT], fp32, bufs=12)
                        a = tmp_pool.tile([128, NT], fp32)
                        u = tmp_pool.tile([128, NT], fp32)
                        pts = [None] * 4

                        def mms_for(pos):
                            pt = psum_pool.tile([128, NT], fp32, tag="pt")
                            pts[pos] = pt
                            for kh in range(3):
                                off = (h0 + kh) * TC
                                # winograd rhs windows are fully contiguous:
                                # flat 486-elem inner dim streams best
                                if mode == "fp8dr":
                                    rhs = dts[n][:, :, pos, off : off + NT]
                                    nc.tensor.matmul(
                                        pt,
                                        wt[:, kh, pos, oc],
                                        rhs,
                                        start=(kh == 0),
                                        stop=(kh == 2),
                                        perf_mode=mybir.MatmulPerfMode.DoubleRow,
                                    )
                                else:
                                    for c in range(2):
                                        rhs = dts[n][:, c, pos, off : off + NT]
                                        nc.tensor.matmul(
                                            pt,
                                            wt[:, kh, pos, oc, c],
                                            rhs,
                                            start=(kh == 0 and c == 0),
                                            stop=(kh == 2 and c == 1),
                                        )

                        mms_for(1)
                        nc.scalar.copy(t1, pts[1])
                        mms_for(2)
                        nc.scalar.copy(t2, pts[2])
                        nc.gpsimd.tensor_tensor(u, t1, t2, mybir.AluOpType.subtract)
                        mms_for(0)
                        nc.vector.tensor_tensor(a, pts[0], t1, mybir.AluOpType.add)
                        nc.vector.tensor_tensor(
                            ot[:, :, 0::2].rearrange("p h w -> p (h w)"),
                            a, t2, mybir.AluOpType.add,
                        )
                        mms_for(3)
                        nc.vector.tensor_tensor(
                            ot[:, :, 1::2].rearrange("p h w -> p (h w)"),
                            u, pts[3], mybir.AluOpType.subtract,
                        )
                        n_group += 1
                        last = n_group == N_GROUPS
                        if last:
                            # split the final store across both rings so the
                            # completion tail is short
                            s = ROWS_PER_CHUNK // 2
                            nc.sync.dma_start(
                                out=out_d[n, oc * 128 : (oc + 1) * 128, h0 : h0 + s, :],
                                in_=ot[:, 0:s, :],
                            )
                            nc.scalar.dma_start(
                                out=out_d[n, oc * 128 : (oc + 1) * 128,
                                          h0 + s : h0 + ROWS_PER_CHUNK, :],
                                in_=ot[:, s:, :],
                            )
                        else:
                            # sync ring is idle after the early loads; keep
                            # stores off scalar so the ACT sequencer only
                            # runs the transform copies
                            nc.sync.dma_start(
                                out=out_d[n, oc * 128 : (oc + 1) * 128,
                                          h0 : h0 + ROWS_PER_CHUNK, :],
                                in_=ot,
                            )
    nc.compile()
    return nc


def get_program(mode="fp8dr"):
    if mode not in _PROGRAM_CACHE:
        _PROGRAM_CACHE[mode] = _build_program(mode)
    return _PROGRAM_CACHE[mode]


def _np_dtype(mode):
    return ml_dtypes.float8_e4m3 if mode == "fp8dr" else ml_dtypes.bfloat16


def prep_weight(weight, mode="fp8dr"):
    """weight [256,256,3,3] OIHW fp32 -> w_sb [128 ki, 3 kh, 4 pos, 2 oc, 2 c, 128 m].

    w1[o,i,kh,pos] = sum_kw G[pos,kw] w[o,i,kh,kw], G = F(2,3) weight transform.
    """
    G = np.array([[1, 0, 0], [0.5, 0.5, 0.5], [0.5, -0.5, 0.5], [0, 0, 1]], np.float32)
    wq = weight.astype(np.int32).astype(np.float32)
    w1 = np.einsum("pk,oihk->oihp", G, wq)  # [o, i, kh, pos]
    w1 = w1.reshape(2, 128, 2, 128, 3, 4)  # [oc, m, c, ki, kh, pos]
    w_sb = np.ascontiguousarray(w1.transpose(3, 4, 5, 0, 2, 1))  # [ki, kh, pos, oc, c, m]
    return w_sb.astype(_np_dtype(mode))


def prep_x_core(x_core, mode="fp8dr"):
    """x_core [IMGS, 256, 56, 56] int32 -> d_sb [128 ki, 2 c, 4 pos, IMGS, 56*27]."""
    xq = np.clip(x_core.astype(np.int32), 0, 7).astype(np.float32)
    xq = xq.reshape(IMGS, 2, 128, H, W)  # [n, c, ki, h, w]
    d0 = xq[..., 0:54:2] - xq[..., 2:56:2]
    d1 = xq[..., 1:55:2] + xq[..., 2:56:2]
    d2 = xq[..., 2:56:2] - xq[..., 1:55:2]
    d3 = xq[..., 1:55:2] - xq[..., 3:56:2]
    d = np.stack([d0, d1, d2, d3], axis=0)  # [pos, n, c, ki, h, tc]
    d_sb = np.ascontiguousarray(d.transpose(3, 2, 0, 1, 4, 5))  # [ki, c, pos, n, h, tc]
    return d_sb.reshape(128, 2, 4, IMGS, DPIX).astype(_np_dtype(mode))


def make_in_maps(x, weight, mode="fp8dr"):
    w_sb = prep_weight(weight, mode)
    return [
        {"x_sb": prep_x_core(x[c * IMGS : (c + 1) * IMGS], mode), "w_sb": w_sb}
        for c in range(N_CORES)
    ]


def kernel(x, weight):
    import time

    from concourse.bass_utils import run_bass_kernel_spmd

    mode = "fp8dr"
    nc = get_program(mode)
    in_maps = make_in_maps(np.asarray(x), np.asarray(weight), mode)
    last_err = None
    for attempt in range(3):
        try:
            res = run_bass_kernel_spmd(nc, in_maps, list(range(N_CORES)))
            break
        except Exception as e:  # transient NRT_EXEC_UNIT_UNRECOVERABLE flakes
            last_err = e
            time.sleep(2.0)
    else:
        raise last_err
    return np.concatenate(
        [res.results[c]["out"] for c in range(N_CORES)], axis=0
    ).astype(np.float32)
